# revision 10
# baseline (speedup 1.0000x reference)
"""Trainium2 Bass kernel for nn_Conv2d_24833500905755 (3x3 conv, B=32,
C_in=64, C_out=128, 56x56, pad 1, with the reference's mismatched
weight-flatten order).

Math: out[b,co,h,w] = sum_{c,di,dj} xpad[b,c,h+di,w+dj] * Wt[c,di*3+dj,co]
with Wt = K.reshape(576, C_OUT).reshape(C_IN, 9, C_OUT).

Data-parallel: 4 images per NeuronCore, 2 images packed on the
128-partition dim (fp16 matmuls, K=64 contraction per half, concurrent
PE row-group tiles). Raw-bass hand-scheduled engine programs.

v2 schedule (vs v1): input + weights split across BOTH hardware DMA
queues by partition half (sync=partitions 0:64, scalar=64:128) with
fine-grained gating (w split per k-group so the first chunk can start
before the whole weight tensor lands); outputs staged per-image in
SBUF and DMA'd in 16-row batches as soon as their chunks are copied
(keeps the queues continuously busy instead of bursty); no trailing
all-engine barrier (the NEFF postamble has its own rendezvous) - only
sync waits for output-DMA completion.

  Sync:   h0 input DMAs, h0 output batch DMAs, final s_out wait
  Scalar: h1 input DMAs, h1 PSUM->SBUF copies, h1 output batch DMAs
  Tensor: 4 junk warm-up pairs + 252 fp16 matmuls
  Vector: h0 PSUM->SBUF copies
"""

from contextlib import ExitStack

import numpy as np

import concourse.bass as bass
import concourse.mybir as mybir
from concourse.bass import BassBlock
from concourse.bass_utils import run_bass_kernel_spmd

B, C_IN, C_OUT, H = 32, 64, 128, 56
KS = 3
N_CORES = 8
BPC = B // N_CORES
HP = H + 2
RCHUNK = 8
NCHUNK = H // RCHUNK          # 7 chunks/image, 14 global chunk-pairs
MM_DT = mybir.dt.float16

# x row pieces per pair: piece i covers rows [XPIECES[i], XPIECES[i+1])
XPIECES = [0, 10, 34, HP]
# chunk ci needs input rows <= ci*8+10; piece gate index per chunk
CHUNK_PIECE = [0, 1, 1, 2, 2, 2, 2]
# output batches (row ranges) per image; last batch kept small for the tail
OBATCH = [(0, 16), (16, 32), (32, 48), (48, 56)]
N_OUT_DMAS = BPC * len(OBATCH)  # per-image batches, one DMA each
# PE clock needs ~4.6us of CONTINUOUS matmul activity to ramp 4/8 -> 8/8;
# any idle gap restarts the ramp. Warm-up pairs run at the slow clock
# (~362ns each) and must bridge from engine start (~7.6us) to first-input
# landing (~10.3us) without a gap.
N_WARMUP_PAIRS = 9
# Junk pairs after the last real matmul: keep the PE clock at 8/8 through
# the output-DMA tail (HAM down-clocks ~2.8us after matmul activity stops,
# which would otherwise slow the sequencer through the NEFF postamble).
N_TAIL_PAIRS = 8


class NoBarrierBlock(BassBlock):
    """BassBlock without the exit-time all-engine barrier/drain: the
    compiler-emitted postamble performs its own all-engine rendezvous
    before touching semaphores, so the extra barrier only adds latency.
    Engine streams simply branch to the common end block."""

    def __exit__(self, exc_type, exc_val, exc_tb):
        if exc_type is None:
            for engine, last_body in self.last_body.items():
                with self.bass.body(
                    last_body, parent=self.bass.cur_bb, allow_existing_parent=True
                ):
                    engine.br(self.end_bb)
            self.bass.switch_bb(self.end_bb)


def build_nc(mm_dt=MM_DT):
    f32 = mybir.dt.float32
    nc = bass.Bass()
    x_ext = nc.declare_dram_parameter("x", [BPC, C_IN, HP, HP], mm_dt, isOutput=False)
    w_ext = nc.declare_dram_parameter("w", [2 * C_IN, KS * KS, C_OUT], mm_dt, isOutput=False)
    out_ext = nc.declare_dram_parameter("out", [BPC, C_OUT, H, H], f32, isOutput=True)

    with ExitStack() as ctx:
        wt = ctx.enter_context(nc.sbuf_tensor("wt", [2 * C_IN, KS * KS, C_OUT], mm_dt))
        xps = [
            ctx.enter_context(nc.sbuf_tensor(f"xp{p}", [2 * C_IN, HP, HP], mm_dt))
            for p in range(2)
        ]
        # per-image output staging: ob[img] = [C_OUT, H, H] f32
        obs = [
            ctx.enter_context(nc.sbuf_tensor(f"ob{b}", [C_OUT, H, H], f32))
            for b in range(BPC)
        ]
        # banks[slot][half] - 8 PSUM banks
        banks = [
            [
                ctx.enter_context(
                    nc.psum_tensor(f"ps_{s}_{h}", [C_OUT, RCHUNK, H], f32)
                )
                for h in range(2)
            ]
            for s in range(4)
        ]
        s_w = ctx.enter_context(nc.semaphore("s_w"))
        # one sem per pair-0 row piece: both queue-halves inc by 16, so a
        # piece is fully resident at >= 32 (a shared counter would race -
        # one queue running two pieces ahead could fake the other's arrival)
        s_x0p = [
            ctx.enter_context(nc.semaphore(f"s_x0p{i}"))
            for i in range(len(XPIECES) - 1)
        ]
        s_x1 = ctx.enter_context(nc.semaphore("s_x1"))
        s_mm = ctx.enter_context(nc.semaphore("s_mm"))
        s_cpv = ctx.enter_context(nc.semaphore("s_cpv"))   # h0 copies (vector)
        s_cph = ctx.enter_context(nc.semaphore("s_cph"))   # h1 copies (scalar)
        s_out = ctx.enter_context(nc.semaphore("s_out"))

        def in_dmas(eng, h):
            """Input DMAs for partition half h on engine eng's queue.
            w as one whole-half DMA (2304B/partition keeps the DMA engines
            near their ~26GB/s per-engine ceiling; k-splitting it makes the
            packets too small and is a net loss)."""
            c0 = h * C_IN
            src0 = x_ext[h : h + 1].rearrange("b c h w -> (b c) h w")
            src1 = x_ext[2 + h : 3 + h].rearrange("b c h w -> (b c) h w")
            eng.dma_start(
                out=wt[c0 : c0 + C_IN, :, :], in_=w_ext[c0 : c0 + C_IN, :, :]
            ).then_inc(s_w, 16)
            for i in range(len(XPIECES) - 1):
                lo, hi = XPIECES[i], XPIECES[i + 1]
                eng.dma_start(
                    out=xps[0][c0 : c0 + C_IN, lo:hi, :], in_=src0[:, lo:hi, :]
                ).then_inc(s_x0p[i], 16)
            eng.dma_start(
                out=xps[1][c0 : c0 + C_IN, :, :], in_=src1[:, :, :]
            ).then_inc(s_x1, 16)

        def out_dmas(eng, h, cp_sem):
            """Output batch DMAs for images of half h, gated on copies."""
            for p in range(2):
                img = 2 * p + h
                dst = out_ext[img : img + 1].rearrange("b c h w -> (b c) h w")
                for (blo, bhi) in OBATCH:
                    last_chunk = p * NCHUNK + (bhi - 1) // RCHUNK
                    eng.wait_ge(cp_sem, last_chunk + 1)
                    eng.dma_start(
                        out=dst[:, blo:bhi, :], in_=obs[img][:, blo:bhi, :]
                    ).then_inc(s_out, 16)

        with NoBarrierBlock(nc, "blk") as block:

            @block.sync
            def _(sync: bass.BassEngine):
                in_dmas(sync, 0)
                out_dmas(sync, 0, s_cpv)
                sync.wait_ge(s_out, 16 * N_OUT_DMAS)

            @block.scalar
            def _(scalar: bass.BassEngine):
                in_dmas(scalar, 1)
                # h1 copies interleaved with h1 output issues (program order
                # on this engine keeps copy -> dma correctly ordered)
                for p in range(2):
                    img = 2 * p + 1
                    dst = out_ext[img : img + 1].rearrange("b c h w -> (b c) h w")
                    bi = 0
                    for ci in range(NCHUNK):
                        c = p * NCHUNK + ci
                        h0 = ci * RCHUNK
                        scalar.wait_ge(s_mm, c + 1)
                        scalar.copy(
                            out=obs[img][:, h0 : h0 + RCHUNK, :],
                            in_=banks[c % 4][1][:],
                        ).then_inc(s_cph, 1)
                        blo, bhi = OBATCH[bi]
                        if h0 + RCHUNK == bhi:
                            scalar.dma_start(
                                out=dst[:, blo:bhi, :], in_=obs[img][:, blo:bhi, :]
                            ).then_inc(s_out, 16)
                            bi += 1

            @block.tensor
            def _(tensor: bass.BassEngine):
                def junk_pairs(n, bank_slot):
                    for wi in range(n):
                        h = wi % 2
                        c0 = h * C_IN
                        tensor.matmul(
                            out=banks[bank_slot][h][:],
                            lhsT=wt[c0 : c0 + C_IN, 0, :],
                            rhs=xps[0][c0 : c0 + C_IN, 0:RCHUNK, 0:H],
                            start=True,
                            stop=True,
                        )

                junk_pairs(2 * N_WARMUP_PAIRS, 3)
                for p in range(2):
                    for ci in range(NCHUNK):
                        c = p * NCHUNK + ci
                        h0 = ci * RCHUNK
                        if p == 0:
                            if ci == 0:
                                tensor.wait_ge(s_w, 32)
                                tensor.wait_ge(s_x0p[0], 32)
                            elif CHUNK_PIECE[ci] > CHUNK_PIECE[ci - 1]:
                                tensor.wait_ge(s_x0p[CHUNK_PIECE[ci]], 32)
                        else:
                            if ci == 0:
                                tensor.wait_ge(s_x1, 32)
                        if c >= 4:
                            # WAR: bank slot c%4 last used by chunk c-4
                            tensor.wait_ge(s_cpv, c - 3)
                            tensor.wait_ge(s_cph, c - 3)
                        for k in range(KS * KS):
                            di, dj = divmod(k, KS)
                            last = k == KS * KS - 1
                            for half in range(2):
                                c0 = half * C_IN
                                mm = tensor.matmul(
                                    out=banks[c % 4][half][:],
                                    lhsT=wt[c0 : c0 + C_IN, k, :],
                                    rhs=xps[p][
                                        c0 : c0 + C_IN,
                                        h0 + di : h0 + di + RCHUNK,
                                        dj : dj + H,
                                    ],
                                    start=(k == 0),
                                    stop=last,
                                )
                                if last and half == 1:
                                    mm.then_inc(s_mm, 1)
                # keep the PE clock up through the output-DMA tail; the
                # last chunk-pair used bank slot 13%4==1, slot 3 was copied
                # ~4 chunks ago so scribbling on it is safe
                junk_pairs(2 * N_TAIL_PAIRS, 3)

            @block.vector
            def _(vector: bass.BassEngine):
                for p in range(2):
                    img = 2 * p
                    for ci in range(NCHUNK):
                        c = p * NCHUNK + ci
                        h0 = ci * RCHUNK
                        vector.wait_ge(s_mm, c + 1)
                        vector.tensor_copy(
                            out=obs[img][:, h0 : h0 + RCHUNK, :],
                            in_=banks[c % 4][0][:],
                        ).then_inc(s_cpv, 1)

    return nc


def _prep_inputs(x, K, mm_dt=MM_DT):
    np_dt = mybir.dt.np(mm_dt)
    x = np.ascontiguousarray(np.asarray(x, dtype=np.float32))
    K = np.ascontiguousarray(np.asarray(K, dtype=np.float32))
    xpad = np.pad(x, ((0, 0), (0, 0), (1, 1), (1, 1))).astype(np_dt)
    Wt = K.reshape(KS * KS * C_IN, C_OUT).reshape(C_IN, KS * KS, C_OUT)
    Wrep = np.ascontiguousarray(np.concatenate([Wt, Wt], axis=0)).astype(np_dt)
    shards = xpad.reshape(N_CORES, BPC, C_IN, HP, HP)
    return [{"x": np.ascontiguousarray(shards[i]), "w": Wrep} for i in range(N_CORES)]


def run(x, K, trace=False, mm_dt=MM_DT):
    nc = build_nc(mm_dt)
    in_maps = _prep_inputs(x, K, mm_dt)
    res = run_bass_kernel_spmd(nc, in_maps, list(range(N_CORES)), trace=trace)
    out = np.concatenate([res.results[i]["out"] for i in range(N_CORES)], axis=0)
    return out, res


def kernel(x, K):
    out, _ = run(x, K, trace=False)
    return out


# revision 11
# speedup vs baseline: 1.0241x; 1.0241x over previous
"""Trainium2 Bass kernel for nn_Conv2d_24833500905755 (3x3 conv, B=32,
C_in=64, C_out=128, 56x56, pad 1, with the reference's mismatched
weight-flatten order).

Math: out[b,co,h,w] = sum_{c,di,dj} xpad[b,c,h+di,w+dj] * Wt[c,di*3+dj,co]
with Wt = K.reshape(576, C_OUT).reshape(C_IN, 9, C_OUT).

Data-parallel: 4 images per NeuronCore, 2 images packed on the
128-partition dim (fp16 matmuls, K=64 contraction per half, concurrent
PE row-group tiles). Raw-bass hand-scheduled engine programs.

v4 schedule notes (from trace analysis):
- DMA completion semaphores are incremented one per engine-slice (16 per
  DMA) and the last 1-2 slices straggle by 1-3us behind the bulk of the
  data. Gates are therefore per queue-half (16 slices, one queue) and the
  h1 (scalar-queue) image stream runs one chunk BEHIND h0, so h0 starts
  on the sync queue's data while h1's stragglers land.
- The PE clock needs ~4.6us of CONTINUOUS matmul activity to ramp to
  8/8; warm-up junk pairs (~373ns each at the slow clock) bridge from
  engine start (~7.2us) to the first gate release (~10.4us). They read a
  dedicated junk SBUF tensor so their SBUF traffic cannot collide with
  the input DMA writes.
- Junk pairs after the last real matmul keep the clock up through the
  output-DMA tail and the NEFF postamble's semaphore-reset chains.
- No trailing all-engine barrier (the postamble rendezvous is enough);
  only sync waits for output-DMA completion.

  Sync:   h0 input DMAs, h0 output batch DMAs, final s_out wait
  Scalar: h1 input DMAs, h1 PSUM->SBUF copies, h1 output batch DMAs
  Tensor: warm-up junk + skewed h0/h1 matmul streams + tail junk
  Vector: h0 PSUM->SBUF copies
"""

from contextlib import ExitStack

import numpy as np

import concourse.bass as bass
import concourse.mybir as mybir
from concourse.bass import BassBlock
from concourse.bass_utils import run_bass_kernel_spmd

B, C_IN, C_OUT, H = 32, 64, 128, 56
KS = 3
N_CORES = 8
BPC = B // N_CORES
HP = H + 2
RCHUNK = 8
NCHUNK = H // RCHUNK          # 7 chunks/image
NCH = 2 * NCHUNK              # 14 chunks per half across both pairs
MM_DT = mybir.dt.float16

# x row pieces per pair-0 image: piece i covers rows [XPIECES[i], XPIECES[i+1])
XPIECES = [0, 10, 34, HP]
# chunk ci needs input rows <= ci*8+10; piece gate index per chunk
CHUNK_PIECE = [0, 1, 1, 2, 2, 2, 2]
# output batches (row ranges) per image; finer at the end so the tail
# drains fast
OBATCH = [(0, 16), (16, 32), (32, 40), (40, 48), (48, 56)]
N_OUT_DMAS = BPC * len(OBATCH)
N_WARMUP_PAIRS = 9
N_TAIL_PAIRS = 19


class NoBarrierBlock(BassBlock):
    """BassBlock without the exit-time all-engine barrier/drain: the
    compiler-emitted postamble performs its own rendezvous before the
    final semaphore teardown, so the extra barrier only adds latency."""

    def __exit__(self, exc_type, exc_val, exc_tb):
        if exc_type is None:
            for engine, last_body in self.last_body.items():
                with self.bass.body(
                    last_body, parent=self.bass.cur_bb, allow_existing_parent=True
                ):
                    engine.br(self.end_bb)
            self.bass.switch_bb(self.end_bb)


def build_nc(mm_dt=MM_DT):
    f32 = mybir.dt.float32
    nc = bass.Bass()
    x_ext = nc.declare_dram_parameter("x", [BPC, C_IN, HP, HP], mm_dt, isOutput=False)
    w_ext = nc.declare_dram_parameter("w", [2 * C_IN, KS * KS, C_OUT], mm_dt, isOutput=False)
    out_ext = nc.declare_dram_parameter("out", [BPC, C_OUT, H, H], f32, isOutput=True)

    with ExitStack() as ctx:
        wt = ctx.enter_context(nc.sbuf_tensor("wt", [2 * C_IN, KS * KS, C_OUT], mm_dt))
        xps = [
            ctx.enter_context(nc.sbuf_tensor(f"xp{p}", [2 * C_IN, HP, HP], mm_dt))
            for p in range(2)
        ]
        junk = ctx.enter_context(nc.sbuf_tensor("junk", [2 * C_IN, RCHUNK, H], mm_dt))
        obs = [
            ctx.enter_context(nc.sbuf_tensor(f"ob{b}", [C_OUT, H, H], f32))
            for b in range(BPC)
        ]
        banks = [
            [
                ctx.enter_context(
                    nc.psum_tensor(f"ps_{s}_{h}", [C_OUT, RCHUNK, H], f32)
                )
                for h in range(2)
            ]
            for s in range(4)
        ]
        # per-queue-half input gates (each inc'd by one DMA: full at >=16)
        s_w = [ctx.enter_context(nc.semaphore(f"s_w{h}")) for h in range(2)]
        s_x0p = [
            [ctx.enter_context(nc.semaphore(f"s_x0p{i}h{h}")) for h in range(2)]
            for i in range(len(XPIECES) - 1)
        ]
        s_x1 = [ctx.enter_context(nc.semaphore(f"s_x1h{h}")) for h in range(2)]
        s_mm = [ctx.enter_context(nc.semaphore(f"s_mm{h}")) for h in range(2)]
        s_cpv = ctx.enter_context(nc.semaphore("s_cpv"))   # h0 copies (vector)
        s_cph = ctx.enter_context(nc.semaphore("s_cph"))   # h1 copies (scalar)
        s_out = ctx.enter_context(nc.semaphore("s_out"))

        def in_dmas(eng, h):
            """Input DMAs for partition half h on engine eng's queue."""
            c0 = h * C_IN
            src0 = x_ext[h : h + 1].rearrange("b c h w -> (b c) h w")
            src1 = x_ext[2 + h : 3 + h].rearrange("b c h w -> (b c) h w")
            eng.dma_start(
                out=wt[c0 : c0 + C_IN, :, :], in_=w_ext[c0 : c0 + C_IN, :, :]
            ).then_inc(s_w[h], 16)
            for i in range(len(XPIECES) - 1):
                lo, hi = XPIECES[i], XPIECES[i + 1]
                eng.dma_start(
                    out=xps[0][c0 : c0 + C_IN, lo:hi, :], in_=src0[:, lo:hi, :]
                ).then_inc(s_x0p[i][h], 16)
            eng.dma_start(
                out=xps[1][c0 : c0 + C_IN, :, :], in_=src1[:, :, :]
            ).then_inc(s_x1[h], 16)

        # global chunk list per half: g -> (pair, chunk-in-image)
        chunks = [(p, ci) for p in range(2) for ci in range(NCHUNK)]

        def chunk_waits(tensor, h, g):
            p, ci = chunks[g]
            if p == 0:
                if ci == 0:
                    tensor.wait_ge(s_w[h], 16)
                    tensor.wait_ge(s_x0p[0][h], 16)
                elif CHUNK_PIECE[ci] > CHUNK_PIECE[ci - 1]:
                    tensor.wait_ge(s_x0p[CHUNK_PIECE[ci]][h], 16)
            else:
                if ci == 0:
                    tensor.wait_ge(s_x1[h], 16)
            if g >= 4:
                # WAR: bank slot g%4 for half h last written by chunk g-4
                tensor.wait_ge(s_cpv if h == 0 else s_cph, g - 3)

        def chunk_mm(tensor, h, g, k):
            p, ci = chunks[g]
            h0r = ci * RCHUNK
            di, dj = divmod(k, KS)
            c0 = h * C_IN
            return tensor.matmul(
                out=banks[g % 4][h][:],
                lhsT=wt[c0 : c0 + C_IN, k, :],
                rhs=xps[p][c0 : c0 + C_IN, h0r + di : h0r + di + RCHUNK, dj : dj + H],
                start=(k == 0),
                stop=(k == KS * KS - 1),
            )

        with NoBarrierBlock(nc, "blk") as block:

            @block.sync
            def _(sync: bass.BassEngine):
                in_dmas(sync, 0)
                for p in range(2):
                    img = 2 * p
                    dst = out_ext[img : img + 1].rearrange("b c h w -> (b c) h w")
                    for (blo, bhi) in OBATCH:
                        last_chunk = p * NCHUNK + (bhi - 1) // RCHUNK
                        sync.wait_ge(s_cpv, last_chunk + 1)
                        sync.dma_start(
                            out=dst[:, blo:bhi, :], in_=obs[img][:, blo:bhi, :]
                        ).then_inc(s_out, 16)
                sync.wait_ge(s_out, 16 * N_OUT_DMAS)

            @block.scalar
            def _(scalar: bass.BassEngine):
                in_dmas(scalar, 1)
                for p in range(2):
                    img = 2 * p + 1
                    dst = out_ext[img : img + 1].rearrange("b c h w -> (b c) h w")
                    bi = 0
                    for ci in range(NCHUNK):
                        g = p * NCHUNK + ci
                        h0r = ci * RCHUNK
                        scalar.wait_ge(s_mm[1], g + 1)
                        scalar.copy(
                            out=obs[img][:, h0r : h0r + RCHUNK, :],
                            in_=banks[g % 4][1][:],
                        ).then_inc(s_cph, 1)
                        blo, bhi = OBATCH[bi]
                        if h0r + RCHUNK == bhi:
                            scalar.dma_start(
                                out=dst[:, blo:bhi, :], in_=obs[img][:, blo:bhi, :]
                            ).then_inc(s_out, 16)
                            bi += 1

            @block.tensor
            def _(tensor: bass.BassEngine):
                def junk_pairs(n):
                    # reads/writes buffers no DMA or copy touches
                    for wi in range(2 * n):
                        h = wi % 2
                        c0 = h * C_IN
                        tensor.matmul(
                            out=banks[2][h][:],
                            lhsT=wt[c0 : c0 + C_IN, 0, :],
                            rhs=junk[c0 : c0 + C_IN, :, :],
                            start=True,
                            stop=True,
                        )

                junk_pairs(N_WARMUP_PAIRS)
                # h0 runs chunk g at slot g; h1 runs chunk g at slot g+1.
                # Interleaved k-loops keep the two PE row-groups paired.
                for slot in range(NCH + 1):
                    g0 = slot if slot < NCH else None
                    g1 = slot - 1 if slot >= 1 else None
                    if g0 is not None:
                        chunk_waits(tensor, 0, g0)
                    if g1 is not None:
                        chunk_waits(tensor, 1, g1)
                    for k in range(KS * KS):
                        last = k == KS * KS - 1
                        if g0 is not None:
                            mm = chunk_mm(tensor, 0, g0, k)
                            if last:
                                mm.then_inc(s_mm[0], 1)
                        if g1 is not None:
                            mm = chunk_mm(tensor, 1, g1, k)
                            if last:
                                mm.then_inc(s_mm[1], 1)
                junk_pairs(N_TAIL_PAIRS)

            @block.vector
            def _(vector: bass.BassEngine):
                for p in range(2):
                    img = 2 * p
                    for ci in range(NCHUNK):
                        g = p * NCHUNK + ci
                        h0r = ci * RCHUNK
                        vector.wait_ge(s_mm[0], g + 1)
                        vector.tensor_copy(
                            out=obs[img][:, h0r : h0r + RCHUNK, :],
                            in_=banks[g % 4][0][:],
                        ).then_inc(s_cpv, 1)

    return nc


def _prep_inputs(x, K, mm_dt=MM_DT):
    np_dt = mybir.dt.np(mm_dt)
    x = np.ascontiguousarray(np.asarray(x, dtype=np.float32))
    K = np.ascontiguousarray(np.asarray(K, dtype=np.float32))
    xpad = np.pad(x, ((0, 0), (0, 0), (1, 1), (1, 1))).astype(np_dt)
    Wt = K.reshape(KS * KS * C_IN, C_OUT).reshape(C_IN, KS * KS, C_OUT)
    Wrep = np.ascontiguousarray(np.concatenate([Wt, Wt], axis=0)).astype(np_dt)
    shards = xpad.reshape(N_CORES, BPC, C_IN, HP, HP)
    return [{"x": np.ascontiguousarray(shards[i]), "w": Wrep} for i in range(N_CORES)]


def run(x, K, trace=False, mm_dt=MM_DT):
    nc = build_nc(mm_dt)
    in_maps = _prep_inputs(x, K, mm_dt)
    res = run_bass_kernel_spmd(nc, in_maps, list(range(N_CORES)), trace=trace)
    out = np.concatenate([res.results[i]["out"] for i in range(N_CORES)], axis=0)
    return out, res


def kernel(x, K):
    out, _ = run(x, K, trace=False)
    return out


# revision 17
# speedup vs baseline: 1.0998x; 1.0739x over previous
"""Trainium2 Bass kernel for nn_Conv2d_24833500905755 (3x3 conv, B=32,
C_in=64, C_out=128, 56x56, pad 1, with the reference's mismatched
weight-flatten order).

Math: out[b,co,h,w] = sum_{c,di,dj} xpad[b,c,h+di,w+dj] * Wt[c,di*3+dj,co]
with Wt = K.reshape(576, C_OUT).reshape(C_IN, 9, C_OUT).

Data-parallel: 4 images per NeuronCore, 2 images packed on the
128-partition dim (fp16 matmuls, K=64 contraction per half, concurrent
PE row-group tiles). Raw-bass hand-scheduled engine programs.

v4 schedule notes (from trace analysis):
- DMA completion semaphores are incremented one per engine-slice (16 per
  DMA) and the last 1-2 slices straggle by 1-3us behind the bulk of the
  data. Gates are therefore per queue-half (16 slices, one queue) and the
  h1 (scalar-queue) image stream runs one chunk BEHIND h0, so h0 starts
  on the sync queue's data while h1's stragglers land.
- The PE clock needs ~4.6us of CONTINUOUS matmul activity to ramp to
  8/8; warm-up junk pairs (~373ns each at the slow clock) bridge from
  engine start (~7.2us) to the first gate release (~10.4us). They read a
  dedicated junk SBUF tensor so their SBUF traffic cannot collide with
  the input DMA writes.
- Junk pairs after the last real matmul keep the clock up through the
  output-DMA tail and the NEFF postamble's semaphore-reset chains.
- No trailing all-engine barrier (the postamble rendezvous is enough);
  only sync waits for output-DMA completion.

  Sync:   h0 input DMAs, h0 output batch DMAs, final s_out wait
  Scalar: h1 input DMAs, h1 PSUM->SBUF copies, h1 output batch DMAs
  Tensor: warm-up junk + skewed h0/h1 matmul streams + tail junk
  Vector: h0 PSUM->SBUF copies
"""

from contextlib import ExitStack

import numpy as np

import concourse.bass as bass
import concourse.mybir as mybir
from concourse.bass import BassBlock
from concourse.bass_utils import run_bass_kernel_spmd

B, C_IN, C_OUT, H = 32, 64, 128, 56
KS = 3
N_CORES = 8
BPC = B // N_CORES
HP = H + 2
RCHUNK = 8
NCHUNK = H // RCHUNK          # 7 chunks/image
NCH = 2 * NCHUNK              # 14 chunks per half across both pairs
MM_DT = mybir.dt.float16

# x row pieces per pair-0 image: piece i covers rows [XPIECES[i], XPIECES[i+1])
XPIECES = [0, 10, 34, HP]
# chunk ci needs input rows <= ci*8+10; piece gate index per chunk
CHUNK_PIECE = [0, 1, 1, 1, 2, 2, 2]
# output batches (row ranges) per image; finer at the end so the tail
# drains fast
OBATCH = [(0, 16), (16, 32), (32, 40), (40, 48), (48, 56)]
N_OUT_DMAS = BPC * len(OBATCH)
N_WARMUP_PAIRS = 11


class NoBarrierBlock(BassBlock):
    """BassBlock without the exit-time all-engine barrier/drain: the
    compiler-emitted postamble performs its own rendezvous before the
    final semaphore teardown, so the extra barrier only adds latency."""

    def __exit__(self, exc_type, exc_val, exc_tb):
        if exc_type is None:
            for engine, last_body in self.last_body.items():
                with self.bass.body(
                    last_body, parent=self.bass.cur_bb, allow_existing_parent=True
                ):
                    engine.br(self.end_bb)
            self.bass.switch_bb(self.end_bb)


def build_nc(mm_dt=MM_DT):
    f32 = mybir.dt.float32
    nc = bass.Bass()
    x_ext = nc.declare_dram_parameter("x", [BPC, C_IN, HP, HP], mm_dt, isOutput=False)
    w_ext = nc.declare_dram_parameter("w", [2 * C_IN, KS * KS, C_OUT], mm_dt, isOutput=False)
    out_ext = nc.declare_dram_parameter("out", [BPC, C_OUT, H, H], f32, isOutput=True)

    with ExitStack() as ctx:
        wt = ctx.enter_context(nc.sbuf_tensor("wt", [2 * C_IN, KS * KS, C_OUT], mm_dt))
        xps = [
            ctx.enter_context(nc.sbuf_tensor(f"xp{p}", [2 * C_IN, HP, HP], mm_dt))
            for p in range(2)
        ]
        junk = ctx.enter_context(nc.sbuf_tensor("junk", [2 * C_IN, RCHUNK, H], mm_dt))
        obs = [
            ctx.enter_context(nc.sbuf_tensor(f"ob{b}", [C_OUT, H, H], f32))
            for b in range(BPC)
        ]
        banks = [
            [
                ctx.enter_context(
                    nc.psum_tensor(f"ps_{s}_{h}", [C_OUT, RCHUNK, H], f32)
                )
                for h in range(2)
            ]
            for s in range(4)
        ]
        # input gates: each inc'd by one single-queue DMA (full at >=16);
        # keeping a gate's DMA alone-per-queue-position avoids conflating
        # the two queues' straggling completion slices
        s_w = ctx.enter_context(nc.semaphore("s_w"))
        s_x0p0 = ctx.enter_context(nc.semaphore("s_x0p0"))
        s_xp = [
            [ctx.enter_context(nc.semaphore(f"s_xp{i}h{h}")) for h in range(2)]
            for i in range(2)  # piece 1, piece 2
        ]
        s_x1 = [ctx.enter_context(nc.semaphore(f"s_x1h{h}")) for h in range(2)]
        s_mm = [ctx.enter_context(nc.semaphore(f"s_mm{h}")) for h in range(2)]
        s_cpv = ctx.enter_context(nc.semaphore("s_cpv"))   # h0 copies (vector)
        s_cph = ctx.enter_context(nc.semaphore("s_cph"))   # h1 copies (scalar)
        s_out = ctx.enter_context(nc.semaphore("s_out"))

        def in_dmas_sync(eng):
            """Q1: whole w first (gates chunk 0), then h0 row pieces."""
            src0 = x_ext[0:1].rearrange("b c h w -> (b c) h w")
            src1 = x_ext[2:3].rearrange("b c h w -> (b c) h w")
            eng.dma_start(out=wt[:, :, :], in_=w_ext[:, :, :]).then_inc(s_w, 16)
            for i in range(2):
                lo, hi = XPIECES[i + 1], XPIECES[i + 2]
                eng.dma_start(
                    out=xps[0][0:C_IN, lo:hi, :], in_=src0[:, lo:hi, :]
                ).then_inc(s_xp[i][0], 16)
            eng.dma_start(
                out=xps[1][0:C_IN, :, :], in_=src1[:, :, :]
            ).then_inc(s_x1[0], 16)

        def in_dmas_scalar(eng):
            """Q10: piece0 for BOTH halves first (gates chunk 0), then h1
            row pieces."""
            src0 = x_ext[0:2].rearrange("b c h w -> (b c) h w")
            s0h1 = x_ext[1:2].rearrange("b c h w -> (b c) h w")
            src1 = x_ext[3:4].rearrange("b c h w -> (b c) h w")
            lo, hi = XPIECES[0], XPIECES[1]
            eng.dma_start(
                out=xps[0][:, lo:hi, :], in_=src0[:, lo:hi, :]
            ).then_inc(s_x0p0, 16)
            for i in range(2):
                lo, hi = XPIECES[i + 1], XPIECES[i + 2]
                eng.dma_start(
                    out=xps[0][C_IN:, lo:hi, :], in_=s0h1[:, lo:hi, :]
                ).then_inc(s_xp[i][1], 16)
            eng.dma_start(
                out=xps[1][C_IN:, :, :], in_=src1[:, :, :]
            ).then_inc(s_x1[1], 16)

        # global chunk list: g -> (pair, chunk-in-image)
        chunks = [(p, ci) for p in range(2) for ci in range(NCHUNK)]

        def chunk_waits(tensor, g):
            p, ci = chunks[g]
            if p == 0:
                if ci == 0:
                    tensor.wait_ge(s_w, 16)
                    tensor.wait_ge(s_x0p0, 16)
                elif CHUNK_PIECE[ci] > CHUNK_PIECE[ci - 1]:
                    pi = CHUNK_PIECE[ci] - 1
                    tensor.wait_ge(s_xp[pi][0], 16)
                    tensor.wait_ge(s_xp[pi][1], 16)
            else:
                if ci == 0:
                    tensor.wait_ge(s_x1[0], 16)
                    tensor.wait_ge(s_x1[1], 16)
            if g >= 4:
                # WAR: bank slot g%4 last written by chunk g-4
                tensor.wait_ge(s_cpv, g - 3)
                tensor.wait_ge(s_cph, g - 3)

        def chunk_mm(tensor, h, g, k):
            p, ci = chunks[g]
            h0r = ci * RCHUNK
            di, dj = divmod(k, KS)
            c0 = h * C_IN
            return tensor.matmul(
                out=banks[g % 4][h][:],
                lhsT=wt[c0 : c0 + C_IN, k, :],
                rhs=xps[p][c0 : c0 + C_IN, h0r + di : h0r + di + RCHUNK, dj : dj + H],
                start=(k == 0),
                stop=(k == KS * KS - 1),
            )

        with NoBarrierBlock(nc, "blk") as block:

            @block.sync
            def _(sync: bass.BassEngine):
                in_dmas_sync(sync)
                for p in range(2):
                    img = 2 * p
                    dst = out_ext[img : img + 1].rearrange("b c h w -> (b c) h w")
                    for (blo, bhi) in OBATCH:
                        last_chunk = p * NCHUNK + (bhi - 1) // RCHUNK
                        sync.wait_ge(s_cpv, last_chunk + 1)
                        sync.dma_start(
                            out=dst[:, blo:bhi, :], in_=obs[img][:, blo:bhi, :]
                        ).then_inc(s_out, 16)
                sync.wait_ge(s_out, 16 * N_OUT_DMAS)

            @block.scalar
            def _(scalar: bass.BassEngine):
                in_dmas_scalar(scalar)
                for p in range(2):
                    img = 2 * p + 1
                    dst = out_ext[img : img + 1].rearrange("b c h w -> (b c) h w")
                    bi = 0
                    for ci in range(NCHUNK):
                        g = p * NCHUNK + ci
                        h0r = ci * RCHUNK
                        scalar.wait_ge(s_mm[1], g + 1)
                        scalar.copy(
                            out=obs[img][:, h0r : h0r + RCHUNK, :],
                            in_=banks[g % 4][1][:],
                        ).then_inc(s_cph, 1)
                        blo, bhi = OBATCH[bi]
                        if h0r + RCHUNK == bhi:
                            scalar.dma_start(
                                out=dst[:, blo:bhi, :], in_=obs[img][:, blo:bhi, :]
                            ).then_inc(s_out, 16)
                            bi += 1

            @block.tensor
            def _(tensor: bass.BassEngine):
                def junk_pairs(n):
                    # reads/writes buffers no DMA or copy touches
                    for wi in range(2 * n):
                        h = wi % 2
                        c0 = h * C_IN
                        tensor.matmul(
                            out=banks[2][h][:],
                            lhsT=wt[c0 : c0 + C_IN, 0, :],
                            rhs=junk[c0 : c0 + C_IN, :, :],
                            start=True,
                            stop=True,
                        )

                junk_pairs(N_WARMUP_PAIRS)
                # fully-paired stream: the clock ramp only advances under
                # full-PE (both row-group) activity, so solo-half slots are
                # a net loss
                for g in range(NCH):
                    chunk_waits(tensor, g)
                    for k in range(KS * KS):
                        last = k == KS * KS - 1
                        for h in range(2):
                            mm = chunk_mm(tensor, h, g, k)
                            if last:
                                mm.then_inc(s_mm[h], 1)

            @block.vector
            def _(vector: bass.BassEngine):
                for p in range(2):
                    img = 2 * p
                    for ci in range(NCHUNK):
                        g = p * NCHUNK + ci
                        h0r = ci * RCHUNK
                        vector.wait_ge(s_mm[0], g + 1)
                        vector.tensor_copy(
                            out=obs[img][:, h0r : h0r + RCHUNK, :],
                            in_=banks[g % 4][0][:],
                        ).then_inc(s_cpv, 1)

    return nc


def _prep_inputs(x, K, mm_dt=MM_DT):
    np_dt = mybir.dt.np(mm_dt)
    x = np.ascontiguousarray(np.asarray(x, dtype=np.float32))
    K = np.ascontiguousarray(np.asarray(K, dtype=np.float32))
    xpad = np.pad(x, ((0, 0), (0, 0), (1, 1), (1, 1))).astype(np_dt)
    Wt = K.reshape(KS * KS * C_IN, C_OUT).reshape(C_IN, KS * KS, C_OUT)
    Wrep = np.ascontiguousarray(np.concatenate([Wt, Wt], axis=0)).astype(np_dt)
    shards = xpad.reshape(N_CORES, BPC, C_IN, HP, HP)
    return [{"x": np.ascontiguousarray(shards[i]), "w": Wrep} for i in range(N_CORES)]


def run(x, K, trace=False, mm_dt=MM_DT):
    nc = build_nc(mm_dt)
    in_maps = _prep_inputs(x, K, mm_dt)
    res = run_bass_kernel_spmd(nc, in_maps, list(range(N_CORES)), trace=trace)
    out = np.concatenate([res.results[i]["out"] for i in range(N_CORES)], axis=0)
    return out, res


def kernel(x, K):
    out, _ = run(x, K, trace=False)
    return out
